# revision 33
# baseline (speedup 1.0000x reference)
"""Trainium2 Bass kernel for nn_MultiHeaded_4080218931880.

Multi-headed attention with the reference's *raw reshape* head split:
    q = from @ Wq + bq                      # (B, F, HD)
    q_r = q.reshape(B, H, D, F)             # raw row-major reshape
    score = einsum('bhdf,bhdt->bhft', q_r, k_r) * alpha
    probs = softmax(score + (1-mask)*NEG, axis=-1)
    out = einsum('bhft,bhdt->bhdf', probs, v_r).reshape(B, H*D, F)

Because the reshape is raw, head h only touches rows [2*D*h, 2*D*(h+1))
of the (F, HD) projection output, and the per-head (D, F) matrix is just
that row block flattened row-major.  So the 32 (b, h) pairs are fully
independent: shard 4 pairs per NeuronCore over 8 cores (pairs of one
batch stay on one core).

Per-core device program, two phases (all matmuls bf16, fp32 PSUM):

Phase P (projections, all 4 pairs):
  q/k/v = x @ W + b with x.T (pre-transposed on host) as stationary and
  W as moving; bias folded in as a K=1 ones-matmul that opens the PSUM
  accumulation group; alpha folded into k's PSUM eviction.  Each block
  is bounced through DRAM to realize the raw (2D, HD) -> (D, 2*HD)
  reshape (rows 2d', 2d'+1 are adjacent in DRAM so the read-back is
  contiguous).  v_r carries an extra ones row (row D) so the PE
  transposes produce v_r.T chunks WITH the ones column that later
  accumulates the softmax denominator.

Phase A (attention, per pair):
  score computed TRANSPOSED in 16 chunks (128 t' x 2048 f'):
  lhsT = k_r chunk (64, 128), rhs = q_r (64, 2048).  exp on ScalarE
  straight out of PSUM into bf16 SBUF tiles.  Context: lhsT = v_r.T
  chunk (128, 65), rhs = exp chunks, accumulated over the 16 chunks in
  PSUM; row 64 of the accumulator is the softmax denominator.
  Normalize via reciprocal + K=1 fp32 broadcast matmul + VectorE mul.
"""

import numpy as np
from contextlib import ExitStack

import concourse.bass as bass
import concourse.bacc as bacc
import concourse.tile as tile
from concourse import mybir
from concourse.bass_utils import run_bass_kernel_spmd
from concourse.masks import make_identity

BF16 = mybir.dt.bfloat16
F32 = mybir.dt.float32
NP_BF16 = mybir.dt.np(mybir.dt.bfloat16)

# Problem dims (hardcoded; harness runs kernel.py standalone).
B, F, T, C = 2, 2048, 2048, 1024
H, D = 16, 64
HD = H * D
ALPHA = 1.0 / np.sqrt(np.float32(D)).astype(np.float32)
NEG = -100000.0
N_CORES = 8
NPAIR = (B * H) // N_CORES  # 4 (b,h) pairs per core
P = 128

REAL_DIMS = dict(npair=NPAIR, c=C, hd=HD, d=D, f=F, t=T)


def _nsl(total, step):
    """Split [0, total) into <=step slices (matmul moving free-dim limit)."""
    return [(s, min(s + step, total)) for s in range(0, total, step)]


def build_program(has_mask=False, has_bias=True, dims=None, exp_bufs=None):
    dm = dims or REAL_DIMS
    npair, c, hd, d, f, t = (
        dm["npair"], dm["c"], dm["hd"], dm["d"], dm["f"], dm["t"],
    )
    bh = 2 * d          # row-block height of x per (b,h) pair
    ncc = c // P        # contraction chunks for projections
    nch = t // P        # t' chunks for attention
    NB = 512            # matmul PSUM-write limit: one 2KB bank (512 f32)
    NF = 512

    nc = bacc.Bacc(None, target_bir_lowering=False, debug=True)
    xfT = nc.declare_dram_parameter("xfT", [npair, c, bh], BF16, isOutput=False)
    xtT = nc.declare_dram_parameter("xtT", [npair, c, bh], BF16, isOutput=False)
    wq = nc.declare_dram_parameter("wq", [c, hd], BF16, isOutput=False)
    wk = nc.declare_dram_parameter("wk", [c, hd], BF16, isOutput=False)
    wv = nc.declare_dram_parameter("wv", [c, hd], BF16, isOutput=False)
    bq = nc.declare_dram_parameter("bq", [1, hd], BF16, isOutput=False)
    bk = nc.declare_dram_parameter("bk", [1, hd], BF16, isOutput=False)
    bv = nc.declare_dram_parameter("bv", [1, hd], BF16, isOutput=False)
    mbT = None
    if has_mask:
        # (1 - mask[b]).T * NEG / ALPHA is NOT needed: alpha lives in k, so
        # the additive bias is exactly (1 - mask[b]).T * NEG.
        mbT = nc.declare_dram_parameter("mbT", [t, f], BF16, isOutput=False)
    out_d = nc.declare_dram_parameter("out", [npair, d, f], F32, isOutput=True)

    with tile.TileContext(nc) as tc, ExitStack() as ctx:
        # ---- pools resident for the whole kernel ----
        const = ctx.enter_context(tc.tile_pool(name="const", bufs=1))
        wpool = ctx.enter_context(tc.tile_pool(name="wpool", bufs=1))
        rqk = ctx.enter_context(tc.tile_pool(name="rqk", bufs=2 * npair))
        vpool = ctx.enter_context(tc.tile_pool(name="vpool", bufs=npair * nch))
        dpool = ctx.enter_context(tc.tile_pool(name="dpool", bufs=3, space="DRAM"))

        if has_bias:
            ones_row = const.tile([1, P], BF16)
            nc.vector.memset(ones_row[:], 1.0)
        # ones row at base partition d: pairs with the reciprocal row (also
        # at partition d) in the K=1 broadcast matmul (matmul requires equal
        # base partitions for lhsT and rhs)
        ones_at_d = const.tile([d + 1, d], F32)
        nc.vector.memset(ones_at_d[d:d + 1, :], 1.0)
        ident = const.tile([d + 1, d + 1], BF16)
        make_identity(nc, ident[:])

        w_s, b_s = {}, {}
        for name, wd, bd in (("q", wq, bq), ("k", wk, bk), ("v", wv, bv)):
            wt = wpool.tile([P, ncc, hd], BF16, tag=f"w{name}")
            nc.sync.dma_start(out=wt[:], in_=wd[:].rearrange("(kc p) n -> p kc n", p=P))
            w_s[name] = wt
            if has_bias:
                bt = wpool.tile([1, hd], BF16, tag=f"b{name}")
                nc.sync.dma_start(out=bt[:], in_=bd[:])
                b_s[name] = bt

        r_all = [{} for _ in range(npair)]
        vones_all = [[] for _ in range(npair)]

        xpool = ctx.enter_context(tc.tile_pool(name="xpool", bufs=2))
        blkpool = ctx.enter_context(tc.tile_pool(name="blkpool", bufs=3))
        rv = ctx.enter_context(tc.tile_pool(name="rv", bufs=2))
        if exp_bufs is None:
            exp_bufs = 8
        epool = ctx.enter_context(tc.tile_pool(name="epool", bufs=exp_bufs))
        opool = ctx.enter_context(tc.tile_pool(name="opool", bufs=2))
        spool = ctx.enter_context(tc.tile_pool(name="spool", bufs=2))
        mpool = None
        if has_mask:
            mpool = ctx.enter_context(tc.tile_pool(name="mpool", bufs=4))
        # PSUM: mix slots (128, hd) f32 [2 banks x 2] serve projections,
        # score halves and the transpose blocks; ctx accumulator [4 banks].
        pp_mix = ctx.enter_context(tc.tile_pool(name="pp_mix", bufs=2, space="PSUM"))
        pp_ctx = ctx.enter_context(tc.tile_pool(name="pp_ctx", bufs=1, space="PSUM"))

        def emit_proj(j):
            """Projections + reshape + v transposes for pair j (generator:
            yields between chunks so the driver can interleave it with the
            previous pair's attention in PE program order)."""
            xf_s = xpool.tile([P, ncc, bh], BF16, tag="xf")
            nc.sync.dma_start(
                out=xf_s[:], in_=xfT[j].rearrange("(kc p) m -> p kc m", p=P)
            )
            xt_s = xpool.tile([P, ncc, bh], BF16, tag="xt")
            nc.sync.dma_start(
                out=xt_s[:], in_=xtT[j].rearrange("(kc p) m -> p kc m", p=P)
            )
            yield
            for name, x_s in (("q", xf_s), ("k", xt_s), ("v", xt_s)):
                pj = pp_mix.tile([bh, hd], F32, tag="mix")
                if has_bias:
                    for ns, ne in _nsl(hd, NB):
                        nc.tensor.matmul(
                            pj[:, ns:ne], ones_row[:, :bh],
                            b_s[name][:, ns:ne],
                            start=True, stop=False,
                        )
                for kc in range(ncc):
                    first = kc == 0 and not has_bias
                    last = kc == ncc - 1
                    for ns, ne in _nsl(hd, NB):
                        nc.tensor.matmul(
                            pj[:, ns:ne], x_s[:, kc, :],
                            w_s[name][:, kc, ns:ne],
                            start=first, stop=last,
                        )
                    if kc % 3 == 2:
                        yield
                blk = blkpool.tile([bh, hd], BF16, tag="blk")
                if name == "k":
                    # fold alpha into k so exp needs no input scale
                    nc.vector.tensor_scalar_mul(blk[:], pj[:], float(ALPHA))
                else:
                    nc.vector.tensor_copy(blk[:], pj[:])
                # DRAM bounce realizes the raw (2d, hd)->(d, 2*hd) reshape:
                # rows 2d', 2d'+1 are adjacent in DRAM, so the read-back is
                # contiguous per partition.
                dsc = dpool.tile([bh, hd], BF16, tag="dsc")
                nc.sync.dma_start(out=dsc[:], in_=blk[:])
                if name == "v":
                    r = rv.tile([d + 1, 2 * hd], BF16, tag="rv")
                else:
                    r = rqk.tile([d, 2 * hd], BF16, tag=f"r{name}")
                nc.sync.dma_start(
                    out=r[0:d, :],
                    in_=dsc[:].rearrange("(d two) n -> d (two n)", two=2),
                )
                r_all[j][name] = r
                yield
            # ones row -> transposes carry the denominator column
            r_v = r_all[j]["v"]
            nc.vector.memset(r_v[d:d + 1, :], 1.0)
            # inner dim padded to d+2 so bf16 PSUM slices stay 4B-aligned
            vt_ps = pp_mix.tile([P, nch, d + 2], BF16, tag="mix")
            for tcb in range(nch):
                nc.tensor.transpose(
                    vt_ps[:, tcb, 0:d + 1],
                    r_v[:, tcb * P:(tcb + 1) * P],
                    ident[:],
                )
                vo = vpool.tile([P, d + 1], BF16, tag="vones")
                nc.vector.tensor_copy(vo[:], vt_ps[:, tcb, 0:d + 1])
                vones_all[j].append(vo)
                if tcb % 4 == 3:
                    yield

        def emit_attn(j):
            """Attention for pair j: per t'-chunk, two score halves + exp,
            then the two ctx matmuls for that chunk (generator: yields per
            chunk)."""
            r_q, r_k = r_all[j]["q"], r_all[j]["k"]
            fh = f // 2
            ps_cx = pp_ctx.tile([d + 1, f], F32, tag="cx")
            for tcb in range(nch):
                exs = []
                for hf in range(2):
                    ps_sc = pp_mix.tile([P, fh], F32, tag="mix")
                    for ns, ne in _nsl(fh, NB):
                        nc.tensor.matmul(
                            ps_sc[:, ns:ne],
                            r_k[:, tcb * P:(tcb + 1) * P],
                            r_q[:, hf * fh + ns:hf * fh + ne],
                            start=True, stop=True,
                        )
                    if has_mask:
                        mt = mpool.tile([P, fh], BF16, tag="mb")
                        nc.sync.dma_start(
                            out=mt[:],
                            in_=mbT[tcb * P:(tcb + 1) * P, hf * fh:(hf + 1) * fh],
                        )
                        nc.vector.tensor_add(ps_sc[:], ps_sc[:], mt[:])
                    ex = epool.tile([P, fh], BF16, tag="exp")
                    nc.scalar.activation(
                        ex[:], ps_sc[:], mybir.ActivationFunctionType.Exp
                    )
                    exs.append(ex)
                # PSUM accumulation groups work on 2KB zero regions (512
                # f32): start/stop must be set on the first/last write of
                # each region, not per slice.
                REG = 512
                for hf in range(2):
                    for ns, ne in _nsl(fh, NB):
                        gs, ge = hf * fh + ns, hf * fh + ne
                        nc.tensor.matmul(
                            ps_cx[:, gs:ge],
                            vones_all[j][tcb][:],
                            exs[hf][:, ns:ne],
                            start=(tcb == 0 and gs % REG == 0),
                            stop=(tcb == nch - 1 and (ge % REG == 0 or ge == f)),
                        )
                yield
            # ctx_t rows 0..d-1 hold the normalized output; row d is
            # scratch for the reciprocal of the softmax denominator.
            ctx_t = opool.tile([d + 1, f], F32, tag="ctx")
            nc.vector.reciprocal(ctx_t[d:d + 1, :], ps_cx[d:d + 1, :])
            # broadcast 1/S to all d partitions via K=1 fp32 matmuls
            bc_sb = spool.tile([d, f], F32, tag="bc")
            for hs, he in _nsl(f, min(fh, 1024)):
                ps_bc = pp_mix.tile([d, min(fh, 1024)], F32, tag="mix")
                for ns, ne in _nsl(he - hs, NF):
                    nc.tensor.matmul(
                        ps_bc[:, ns:ne], ones_at_d[d:d + 1, :],
                        ctx_t[d:d + 1, hs + ns:hs + ne],
                        start=True, stop=True,
                    )
                nc.vector.tensor_copy(bc_sb[:, hs:he], ps_bc[:, 0:he - hs])
            nc.vector.tensor_mul(ctx_t[0:d, :], ps_cx[0:d, :], bc_sb[:])
            nc.sync.dma_start(out=out_d[j], in_=ctx_t[0:d, :])
            yield

        # software pipeline: pair j's attention interleaved (in program
        # order, hence in each engine's instruction stream) with pair j+1's
        # projections.
        for _ in emit_proj(0):
            pass
        for j in range(npair):
            pg = emit_proj(j + 1) if j + 1 < npair else None
            for _ in emit_attn(j):
                if pg is not None:
                    next(pg, None)
            if pg is not None:
                for _ in pg:
                    pass

    nc.finalize()
    return nc


_PROGRAM_CACHE = {}
TRACE = False
LAST_RESULTS = None


def _get_program(has_mask, has_bias):
    key = (has_mask, has_bias)
    if key not in _PROGRAM_CACHE:
        _PROGRAM_CACHE[key] = build_program(has_mask=has_mask, has_bias=has_bias)
    return _PROGRAM_CACHE[key]


def kernel(**inputs):
    from_tensor = np.asarray(inputs["from_tensor"], np.float32)
    to_tensor = np.asarray(inputs["to_tensor"], np.float32)
    mask = np.asarray(inputs["mask"], np.float32)
    wq = np.ascontiguousarray(np.asarray(inputs["Wq"], np.float32).astype(NP_BF16))
    wk = np.ascontiguousarray(np.asarray(inputs["Wk"], np.float32).astype(NP_BF16))
    wv = np.ascontiguousarray(np.asarray(inputs["Wv"], np.float32).astype(NP_BF16))
    bqv = np.asarray(inputs["bq"], np.float32).astype(NP_BF16).reshape(1, HD)
    bkv = np.asarray(inputs["bk"], np.float32).astype(NP_BF16).reshape(1, HD)
    bvv = np.asarray(inputs["bv"], np.float32).astype(NP_BF16).reshape(1, HD)

    mb = (1.0 - mask) * NEG  # (B, F, T) additive mask bias
    has_mask = bool(np.any(mb != 0.0))
    has_bias = bool(
        np.any(inputs["bq"]) or np.any(inputs["bk"]) or np.any(inputs["bv"])
    )
    nc = _get_program(has_mask, has_bias)

    bh = 2 * D
    in_maps = []
    for core in range(N_CORES):
        pairs = [4 * core + jj for jj in range(NPAIR)]
        b = pairs[0] // H
        xf = np.stack(
            [
                np.ascontiguousarray(
                    from_tensor[p // H, (p % H) * bh:(p % H + 1) * bh, :].T
                ).astype(NP_BF16)
                for p in pairs
            ]
        )
        xt = np.stack(
            [
                np.ascontiguousarray(
                    to_tensor[p // H, (p % H) * bh:(p % H + 1) * bh, :].T
                ).astype(NP_BF16)
                for p in pairs
            ]
        )
        m = {
            "xfT": xf, "xtT": xt,
            "wq": wq, "wk": wk, "wv": wv,
            "bq": bqv, "bk": bkv, "bv": bvv,
        }
        if has_mask:
            m["mbT"] = np.ascontiguousarray(mb[b].T).astype(NP_BF16)
        in_maps.append(m)

    res = run_bass_kernel_spmd(
        nc, in_maps, core_ids=list(range(N_CORES)), trace=TRACE
    )
    global LAST_RESULTS
    LAST_RESULTS = res

    out = np.empty((B, HD, F), np.float32)
    for core in range(N_CORES):
        o = res.results[core]["out"]
        for jj in range(NPAIR):
            p = 4 * core + jj
            b, h = p // H, p % H
            out[b, h * D:(h + 1) * D, :] = o[jj]
    return out


# revision 44
# speedup vs baseline: 1.1135x; 1.1135x over previous
"""Trainium2 Bass kernel for nn_MultiHeaded_4080218931880.

Multi-headed attention with the reference's *raw reshape* head split:
    q = from @ Wq + bq                      # (B, F, HD)
    q_r = q.reshape(B, H, D, F)             # raw row-major reshape
    score = einsum('bhdf,bhdt->bhft', q_r, k_r) * alpha
    probs = softmax(score + (1-mask)*NEG, axis=-1)
    out = einsum('bhft,bhdt->bhdf', probs, v_r).reshape(B, H*D, F)

Because the reshape is raw, head h only touches rows [2*D*h, 2*D*(h+1))
of the (F, HD) projection output, and the per-head (D, F) matrix is just
that row block flattened row-major.  So the 32 (b, h) pairs are fully
independent: shard 4 pairs per NeuronCore over 8 cores (pairs of one
batch stay on one core).

Per-core device program, two phases (all matmuls bf16, fp32 PSUM):

Phase P (projections, all 4 pairs):
  q/k/v = x @ W + b with x.T (pre-transposed on host) as stationary and
  W as moving; bias folded in as a K=1 ones-matmul that opens the PSUM
  accumulation group; alpha folded into k's PSUM eviction.  Each block
  is bounced through DRAM to realize the raw (2D, HD) -> (D, 2*HD)
  reshape (rows 2d', 2d'+1 are adjacent in DRAM so the read-back is
  contiguous).  v_r carries an extra ones row (row D) so the PE
  transposes produce v_r.T chunks WITH the ones column that later
  accumulates the softmax denominator.

Phase A (attention, per pair):
  score computed TRANSPOSED in 16 chunks (128 t' x 2048 f'):
  lhsT = k_r chunk (64, 128), rhs = q_r (64, 2048).  exp on ScalarE
  straight out of PSUM into bf16 SBUF tiles.  Context: lhsT = v_r.T
  chunk (128, 65), rhs = exp chunks, accumulated over the 16 chunks in
  PSUM; row 64 of the accumulator is the softmax denominator.
  Normalize via reciprocal + K=1 fp32 broadcast matmul + VectorE mul.
"""

import numpy as np
from contextlib import ExitStack

import concourse.bass as bass
import concourse.bacc as bacc
import concourse.tile as tile
from concourse import mybir
from concourse.bass_utils import run_bass_kernel_spmd
from concourse.masks import make_identity

BF16 = mybir.dt.bfloat16
F32 = mybir.dt.float32
NP_BF16 = mybir.dt.np(mybir.dt.bfloat16)

# Problem dims (hardcoded; harness runs kernel.py standalone).
B, F, T, C = 2, 2048, 2048, 1024
H, D = 16, 64
HD = H * D
ALPHA = 1.0 / np.sqrt(np.float32(D)).astype(np.float32)
NEG = -100000.0
N_CORES = 8
NPAIR = (B * H) // N_CORES  # 4 (b,h) pairs per core
P = 128

REAL_DIMS = dict(npair=NPAIR, c=C, hd=HD, d=D, f=F, t=T)


def _nsl(total, step):
    """Split [0, total) into <=step slices (matmul moving free-dim limit)."""
    return [(s, min(s + step, total)) for s in range(0, total, step)]


def build_program(has_mask=False, has_bias=True, dims=None, exp_bufs=None):
    dm = dims or REAL_DIMS
    npair, c, hd, d, f, t = (
        dm["npair"], dm["c"], dm["hd"], dm["d"], dm["f"], dm["t"],
    )
    bh = 2 * d          # row-block height of x per (b,h) pair
    ncc = c // P        # contraction chunks for projections
    nch = t // P        # t' chunks for attention
    NB = 512            # matmul PSUM-write limit: one 2KB bank (512 f32)
    NF = 512

    nc = bacc.Bacc(None, target_bir_lowering=False, debug=True)
    xfT = nc.declare_dram_parameter("xfT", [npair, c, bh], BF16, isOutput=False)
    xtT = nc.declare_dram_parameter("xtT", [npair, c, bh], BF16, isOutput=False)
    wq = nc.declare_dram_parameter("wq", [c, hd], BF16, isOutput=False)
    wk = nc.declare_dram_parameter("wk", [c, hd], BF16, isOutput=False)
    wv = nc.declare_dram_parameter("wv", [c, hd], BF16, isOutput=False)
    bq = nc.declare_dram_parameter("bq", [1, hd], BF16, isOutput=False)
    bk = nc.declare_dram_parameter("bk", [1, hd], BF16, isOutput=False)
    bv = nc.declare_dram_parameter("bv", [1, hd], BF16, isOutput=False)
    mbT = None
    if has_mask:
        # (1 - mask[b]).T * NEG / ALPHA is NOT needed: alpha lives in k, so
        # the additive bias is exactly (1 - mask[b]).T * NEG.
        mbT = nc.declare_dram_parameter("mbT", [t, f], BF16, isOutput=False)
    out_d = nc.declare_dram_parameter("out", [npair, d, f], F32, isOutput=True)

    with tile.TileContext(nc) as tc, ExitStack() as ctx:
        # ---- pools resident for the whole kernel ----
        const = ctx.enter_context(tc.tile_pool(name="const", bufs=1))
        wpool = ctx.enter_context(tc.tile_pool(name="wpool", bufs=1))
        rqk = ctx.enter_context(tc.tile_pool(name="rqk", bufs=2 * npair))
        vpool = ctx.enter_context(tc.tile_pool(name="vpool", bufs=npair * nch))
        dpool = ctx.enter_context(tc.tile_pool(name="dpool", bufs=3, space="DRAM"))

        if has_bias:
            ones_row = const.tile([1, P], BF16)
            nc.vector.memset(ones_row[:], 1.0)
        # ones row at base partition d: pairs with the reciprocal row (also
        # at partition d) in the K=1 broadcast matmul (matmul requires equal
        # base partitions for lhsT and rhs); bf16 so the broadcast streams
        # at 1 cycle/row instead of fp32's 4
        ones_at_d = const.tile([d + 1, d], BF16)
        nc.vector.memset(ones_at_d[d:d + 1, :], 1.0)
        ident = const.tile([d + 1, d + 1], BF16)
        make_identity(nc, ident[:])

        w_s, b_s = {}, {}

        def load_weights():
            for name, wd, bd in (("q", wq, bq), ("k", wk, bk), ("v", wv, bv)):
                wt = wpool.tile([P, ncc, hd], BF16, tag=f"w{name}")
                # per-chunk loads so the first projection matmul only waits
                # for one 256KB transfer, not the whole 2MB weight
                wdr = wd[:].rearrange("(kc p) n -> p kc n", p=P)
                for kc in range(ncc):
                    nc.sync.dma_start(out=wt[:, kc, :], in_=wdr[:, kc, :])
                w_s[name] = wt
                if has_bias:
                    bt = wpool.tile([1, hd], BF16, tag=f"b{name}")
                    nc.sync.dma_start(out=bt[:], in_=bd[:])
                    b_s[name] = bt

        r_all = [{} for _ in range(npair)]
        vones_all = [[] for _ in range(npair)]
        cx_hold = {}
        fh = f // 2

        xpool = ctx.enter_context(tc.tile_pool(name="xpool", bufs=2))
        blkpool = ctx.enter_context(tc.tile_pool(name="blkpool", bufs=3))
        rv = ctx.enter_context(tc.tile_pool(name="rv", bufs=2))
        if exp_bufs is None:
            exp_bufs = 12
        epool = ctx.enter_context(tc.tile_pool(name="epool", bufs=exp_bufs))
        opool = ctx.enter_context(tc.tile_pool(name="opool", bufs=2))
        spool = ctx.enter_context(tc.tile_pool(name="spool", bufs=1))
        mpool = None
        if has_mask:
            mpool = ctx.enter_context(tc.tile_pool(name="mpool", bufs=4))
        # PSUM: mix slots (128, hd) f32 [2 banks x 2] serve projections,
        # score halves and the transpose blocks; ctx accumulator [4 banks].
        pp_mix = ctx.enter_context(tc.tile_pool(name="pp_mix", bufs=2, space="PSUM"))
        pp_ctx = ctx.enter_context(tc.tile_pool(name="pp_ctx", bufs=1, space="PSUM"))

        def emit_proj(j):
            """Projections + reshape + v transposes for pair j (generator:
            yields between chunks so the driver can interleave it with the
            previous pair's attention in PE program order)."""
            xf_s = xpool.tile([P, ncc, bh], BF16, tag="xf")
            nc.sync.dma_start(
                out=xf_s[:], in_=xfT[j].rearrange("(kc p) m -> p kc m", p=P)
            )
            xt_s = xpool.tile([P, ncc, bh], BF16, tag="xt")
            nc.sync.dma_start(
                out=xt_s[:], in_=xtT[j].rearrange("(kc p) m -> p kc m", p=P)
            )
            yield
            for name, x_s in (("q", xf_s), ("k", xt_s), ("v", xt_s)):
                pj = pp_mix.tile([bh, hd], F32, tag="mix")
                if has_bias:
                    for ns, ne in _nsl(hd, NB):
                        nc.tensor.matmul(
                            pj[:, ns:ne], ones_row[:, :bh],
                            b_s[name][:, ns:ne],
                            start=True, stop=False,
                        )
                for kc in range(ncc):
                    first = kc == 0 and not has_bias
                    last = kc == ncc - 1
                    for ns, ne in _nsl(hd, NB):
                        nc.tensor.matmul(
                            pj[:, ns:ne], x_s[:, kc, :],
                            w_s[name][:, kc, ns:ne],
                            start=first, stop=last,
                        )
                    if kc % 3 == 2:
                        yield
                blk = blkpool.tile([bh, hd], BF16, tag="blk")
                if name == "k":
                    # fold alpha into k so exp needs no input scale
                    nc.vector.tensor_scalar_mul(blk[:], pj[:], float(ALPHA))
                else:
                    nc.vector.tensor_copy(blk[:], pj[:])
                # DRAM bounce realizes the raw (2d, hd)->(d, 2*hd) reshape:
                # rows 2d', 2d'+1 are adjacent in DRAM, so the read-back is
                # contiguous per partition.
                dsc = dpool.tile([bh, hd], BF16, tag="dsc")
                nc.sync.dma_start(out=dsc[:], in_=blk[:])
                if name == "v":
                    r = rv.tile([d + 1, 2 * hd], BF16, tag="rv")
                else:
                    r = rqk.tile([d, 2 * hd], BF16, tag=f"r{name}")
                nc.sync.dma_start(
                    out=r[0:d, :],
                    in_=dsc[:].rearrange("(d two) n -> d (two n)", two=2),
                )
                r_all[j][name] = r
                yield
            # ones row -> transposes carry the denominator column
            r_v = r_all[j]["v"]
            nc.vector.memset(r_v[d:d + 1, :], 1.0)
            # transposes in 4-chunk blocks so each PSUM mix-slot hold is
            # short (a long hold single-buffers the score pipeline);
            # inner dim padded to d+2 so bf16 PSUM slices stay 4B-aligned
            grp = 4
            for tg in range(0, nch, grp):
                gn = min(grp, nch - tg)
                vt_ps = pp_mix.tile([P, grp, d + 2], BF16, tag="mix")
                for ti in range(gn):
                    tcb = tg + ti
                    nc.tensor.transpose(
                        vt_ps[:, ti, 0:d + 1],
                        r_v[:, tcb * P:(tcb + 1) * P],
                        ident[:],
                    )
                    vo = vpool.tile([P, d + 1], BF16, tag="vones")
                    nc.vector.tensor_copy(vo[:], vt_ps[:, ti, 0:d + 1])
                    vones_all[j].append(vo)
                yield

        def emit_attn(j):
            """Attention for pair j: per t'-chunk, two score halves + exp,
            then the two ctx matmuls for that chunk (generator: yields per
            chunk)."""
            r_q, r_k = r_all[j]["q"], r_all[j]["k"]
            ps_cx = pp_ctx.tile([d + 1, f], F32, tag="cx")
            for tcb in range(nch):
                exs = []
                for hf in range(2):
                    ps_sc = pp_mix.tile([P, fh], F32, tag="mix")
                    for ns, ne in _nsl(fh, NB):
                        nc.tensor.matmul(
                            ps_sc[:, ns:ne],
                            r_k[:, tcb * P:(tcb + 1) * P],
                            r_q[:, hf * fh + ns:hf * fh + ne],
                            start=True, stop=True,
                        )
                    if has_mask:
                        mt = mpool.tile([P, fh], BF16, tag="mb")
                        nc.sync.dma_start(
                            out=mt[:],
                            in_=mbT[tcb * P:(tcb + 1) * P, hf * fh:(hf + 1) * fh],
                        )
                        nc.vector.tensor_add(ps_sc[:], ps_sc[:], mt[:])
                    ex = epool.tile([P, fh], BF16, tag="exp")
                    nc.scalar.activation(
                        ex[:], ps_sc[:], mybir.ActivationFunctionType.Exp
                    )
                    exs.append(ex)
                # PSUM accumulation groups work on 2KB zero regions (512
                # f32): start/stop must be set on the first/last write of
                # each region, not per slice.
                REG = 512
                for hf in range(2):
                    for ns, ne in _nsl(fh, NB):
                        gs, ge = hf * fh + ns, hf * fh + ne
                        nc.tensor.matmul(
                            ps_cx[:, gs:ge],
                            vones_all[j][tcb][:],
                            exs[hf][:, ns:ne],
                            start=(tcb == 0 and gs % REG == 0),
                            stop=(tcb == nch - 1 and (ge % REG == 0 or ge == f)),
                        )
                yield
            # ctx_t rows 0..d-1 hold the normalized output; row d is
            # scratch for the reciprocal of the softmax denominator.
            # eagerly evacuate the accumulator to SBUF so the 4-bank ctx
            # PSUM slot frees for the next pair; normalization happens
            # lazily, interleaved with the next pair's attention.
            cx_sb = opool.tile([d + 1, f], F32, tag="ctx")
            nc.vector.tensor_copy(cx_sb[:], ps_cx[:])
            cx_hold[j] = cx_sb
            yield

        def emit_norm(j):
            """Normalize pair j's evacuated accumulator and store it."""
            cx_sb = cx_hold[j]
            nc.vector.reciprocal(cx_sb[d:d + 1, :], cx_sb[d:d + 1, :])
            # bf16 copy of 1/S (same partition), then K=1 bf16 broadcast
            # matmuls; the ~0.4% bf16 error on 1/S is well inside budget
            rc_bf = spool.tile([d + 1, f], BF16, tag="rcb")
            nc.vector.tensor_copy(rc_bf[d:d + 1, :], cx_sb[d:d + 1, :])
            yield
            bc_sb = spool.tile([d, f], F32, tag="bc")
            for hs, he in _nsl(f, min(fh, 1024)):
                ps_bc = pp_mix.tile([d, min(fh, 1024)], F32, tag="mix")
                for ns, ne in _nsl(he - hs, NB):
                    nc.tensor.matmul(
                        ps_bc[:, ns:ne], ones_at_d[d:d + 1, :],
                        rc_bf[d:d + 1, hs + ns:hs + ne],
                        start=True, stop=True,
                    )
                nc.vector.tensor_copy(bc_sb[:, hs:he], ps_bc[:, 0:he - hs])
                yield
            nc.vector.tensor_mul(cx_sb[0:d, :], cx_sb[0:d, :], bc_sb[:])
            nc.sync.dma_start(out=out_d[j], in_=cx_sb[0:d, :])
            yield

        # software pipeline: pair j's attention interleaved (in program
        # order, hence in each engine's instruction stream) with pair j+1's
        # projections and pair j-1's normalization.
        pg0 = emit_proj(0)
        next(pg0)        # pair-0 x loads issue before the weight DMAs
        load_weights()
        for _ in pg0:
            pass
        ng = None
        for j in range(npair):
            pg = emit_proj(j + 1) if j + 1 < npair else None
            for _ in emit_attn(j):
                if pg is not None:
                    next(pg, None)
                if ng is not None:
                    next(ng, None)
            if pg is not None:
                for _ in pg:
                    pass
            if ng is not None:
                for _ in ng:
                    pass
            ng = emit_norm(j)
        for _ in ng:
            pass

    nc.finalize()
    return nc


_PROGRAM_CACHE = {}
TRACE = False
LAST_RESULTS = None


def _get_program(has_mask, has_bias):
    key = (has_mask, has_bias)
    if key not in _PROGRAM_CACHE:
        _PROGRAM_CACHE[key] = build_program(has_mask=has_mask, has_bias=has_bias)
    return _PROGRAM_CACHE[key]


def kernel(**inputs):
    from_tensor = np.asarray(inputs["from_tensor"], np.float32)
    to_tensor = np.asarray(inputs["to_tensor"], np.float32)
    mask = np.asarray(inputs["mask"], np.float32)
    wq = np.ascontiguousarray(np.asarray(inputs["Wq"], np.float32).astype(NP_BF16))
    wk = np.ascontiguousarray(np.asarray(inputs["Wk"], np.float32).astype(NP_BF16))
    wv = np.ascontiguousarray(np.asarray(inputs["Wv"], np.float32).astype(NP_BF16))
    bqv = np.asarray(inputs["bq"], np.float32).astype(NP_BF16).reshape(1, HD)
    bkv = np.asarray(inputs["bk"], np.float32).astype(NP_BF16).reshape(1, HD)
    bvv = np.asarray(inputs["bv"], np.float32).astype(NP_BF16).reshape(1, HD)

    mb = (1.0 - mask) * NEG  # (B, F, T) additive mask bias
    has_mask = bool(np.any(mb != 0.0))
    has_bias = bool(
        np.any(inputs["bq"]) or np.any(inputs["bk"]) or np.any(inputs["bv"])
    )
    nc = _get_program(has_mask, has_bias)

    bh = 2 * D
    in_maps = []
    for core in range(N_CORES):
        pairs = [4 * core + jj for jj in range(NPAIR)]
        b = pairs[0] // H
        xf = np.stack(
            [
                np.ascontiguousarray(
                    from_tensor[p // H, (p % H) * bh:(p % H + 1) * bh, :].T
                ).astype(NP_BF16)
                for p in pairs
            ]
        )
        xt = np.stack(
            [
                np.ascontiguousarray(
                    to_tensor[p // H, (p % H) * bh:(p % H + 1) * bh, :].T
                ).astype(NP_BF16)
                for p in pairs
            ]
        )
        m = {
            "xfT": xf, "xtT": xt,
            "wq": wq, "wk": wk, "wv": wv,
            "bq": bqv, "bk": bkv, "bv": bvv,
        }
        if has_mask:
            m["mbT"] = np.ascontiguousarray(mb[b].T).astype(NP_BF16)
        in_maps.append(m)

    res = run_bass_kernel_spmd(
        nc, in_maps, core_ids=list(range(N_CORES)), trace=TRACE
    )
    global LAST_RESULTS
    LAST_RESULTS = res

    out = np.empty((B, HD, F), np.float32)
    for core in range(N_CORES):
        o = res.results[core]["out"]
        for jj in range(NPAIR):
            p = 4 * core + jj
            b, h = p // H, p % H
            out[b, h * D:(h + 1) * D, :] = o[jj]
    return out
